# revision 21
# baseline (speedup 1.0000x reference)
"""Trainium2 Bass kernel for NeighborhoodNormalization.

Math: the reference builds a per-point homogeneous transform
T = [[ux,-uy,0,px],[uy,ux,0,py],[0,0,1,pz],[0,0,0,1]] (u = p/||p||),
inverts it, and applies it to 64 neighbors per point.  Closed form with
r2 = px^2+py^2, n = ||p||, a = n/r2, cx = px*a, cy = py*a, d = q - p:

    out.x =  cx*dx + cy*dy
    out.y =  cx*dy - cy*dx
    out.z =  dz

Strategy (memory-bound, tolerance 2e-2 allows bf16):
  * Host converts neighborhoods to bf16 in a per-core SBUF layout
    [128 partitions, 128 cols, 192] whose inner 192 elems are the 64
    (qx,qy) interleaved pairs followed by the 64 qz -> halves HBM traffic,
    every group DMA is one contiguous 6 KB run per partition (415 GB/s
    measured), and the xy math runs as MERGED 2048-elem pair ops:
        dxy  = qxy + (-px,-py)        m14  = dxy * (cx,cx)
        m23s = swap(dxy) * (cy,-cy)   oxy  = m14 + m23s
    where swap() is a step -1 inner view (allowed by the DVE 2x perf mode)
    so DVE does 3 wide ops + the in-place z subtraction per group.
  * The final oxy sum runs on the otherwise-idle TensorE as identity-matmul
    PSUM accumulations; ACT copies PSUM back into the group tile, so one
    DMA per group moves all planes out.  The LAST group sums on DVE and
    stores via the idle sync ring so the tail skips the PE->ACT drain.
  * Per-point values enter as [P,T,2] bf16 pair tiles viewed with
    (..)(K:0)(2:1) broadcast APs (keeps the 16-bit 2x DVE mode).
  * GpSimd does NOTHING: its elementwise ops share an SBUF port with DVE
    and degrade DVE ~3.5x (measured); SWDGE DMA-accumulate was abandoned
    (half-rate CCE descriptors that break past 2048 elems/run).

Sharding: pure data parallel over N=8192 points across 8 cores.
Per-core layout: 16384 points = 128 partitions x 128 columns, partition
p = b*8 + s holds points with local n = s*128 + t.
"""

import sys

if "/opt/trn_rl_repo" not in sys.path:
    sys.path.insert(0, "/opt/trn_rl_repo")

import numpy as np
from ml_dtypes import bfloat16

import concourse.bass as bass
import concourse.bacc as bacc
import concourse.mybir as mybir
from concourse.bass import MemorySpace
from concourse.tile import TileContext
from concourse.bass_utils import run_bass_kernel_spmd

B = 16
N = 8192
K = 64
NCORES = 8
NLOC = N // NCORES  # 1024 points per core
P = 128             # SBUF partitions
S = NLOC // P       # 8 partition sub-blocks per batch entry
T = (B * NLOC) // P  # 128 point-columns per partition
G = 16              # columns per group
NG = T // G
F2 = 2 * G * K      # xy elems per group (2048)
FZ = G * K          # z elems per group (1024)
ROW = 3 * K         # 192 elems per column (128 xy-interleaved + 64 z)
MM = 512            # moving free-dim max per matmul

F32 = mybir.dt.float32
BF16 = mybir.dt.bfloat16
OP = mybir.AluOpType
AF = mybir.ActivationFunctionType

_CACHE = {}


def _build_nc():
    nc = bacc.Bacc(None, target_bir_lowering=False)

    pts = nc.declare_dram_parameter("points", [B, NLOC, 3], F32, isOutput=False)
    nb = nc.declare_dram_parameter("nbh", [P, T, ROW], BF16, isOutput=False)
    ident = nc.declare_dram_parameter("ident", [P, P], BF16, isOutput=False)
    out = nc.declare_dram_parameter("outh", [P, T, ROW], BF16, isOutput=True)

    nbr = nb[:]
    outr = out[:]
    ptsr = pts[:].rearrange("b (s t) c -> (b s) (t c)", s=S)

    with TileContext(nc) as tc:
        with tc.tile_pool(name="const", bufs=1) as cpool, \
             tc.tile_pool(name="io", bufs=1) as iopool, \
             tc.tile_pool(name="tmp", bufs=3) as tmppool, \
             tc.tile_pool(name="ps", bufs=2, space=MemorySpace.PSUM) as pspool:

            pts_sb = cpool.tile([P, T * 3], F32, tag="pts")
            nc.sync.dma_start(out=pts_sb[:], in_=ptsr)
            id_sb = cpool.tile([P, P], BF16, tag="id")
            nc.sync.dma_start(out=id_sb[:], in_=ident[:])

            pv = pts_sb[:].rearrange("p (t c) -> p t c", c=3)
            px = pv[:, :, 0]
            py = pv[:, :, 1]
            pz = pv[:, :, 2]

            def ctile(tag, dtype=F32, shape=None):
                return cpool.tile(shape or [P, T], dtype, tag=tag, name=tag)

            # (-px, -py) pairs and (-pz, -pz) pairs (need only pts)
            nxy2 = ctile("nxy2", BF16, [P, T, 2])
            nzd = ctile("nzd", BF16, [P, T, 2])
            nc.vector.tensor_scalar_mul(
                out=nxy2[:], in0=pv[:, :, 0:2], scalar1=-1.0)
            nc.vector.tensor_scalar_mul(
                out=nzd[:], in0=pv[:, :, 2, None].broadcast_to([P, T, 2]),
                scalar1=-1.0,
            )

            t1 = ctile("t1")
            t2 = ctile("t2")
            r2 = ctile("r2")
            n2 = ctile("n2")
            nn = ctile("nn")
            ir2 = ctile("ir2")
            aa = ctile("aa")

            nc.vector.tensor_mul(out=t1[:], in0=px, in1=px)
            nc.vector.tensor_mul(out=t2[:], in0=py, in1=py)
            nc.vector.tensor_add(out=r2[:], in0=t1[:], in1=t2[:])
            nc.vector.tensor_mul(out=t1[:], in0=pz, in1=pz)
            nc.vector.tensor_add(out=n2[:], in0=r2[:], in1=t1[:])
            nc.scalar.sqrt(out=nn[:], in_=n2[:])
            # ~18 correct bits, 5x faster than reciprocal(); bf16 keeps 8.
            # randn points make r2=0/denorm/inf impossible.
            nc.vector.reciprocal_approx_fast(out=ir2[:], in_=r2[:])
            nc.vector.tensor_mul(out=aa[:], in0=nn[:], in1=ir2[:])

            # coefficient pair tiles, written directly as bf16 pairs
            cxd = ctile("cxd", BF16, [P, T, 2])   # (cx, cx)
            cyn = ctile("cyn", BF16, [P, T, 2])   # (cy, -cy)
            aasn = ctile("aasn", F32, [P, T, 2])  # (aa, -aa)
            aab = aa[:, :, None].broadcast_to([P, T, 2])
            nc.vector.tensor_copy(aasn[:, :, 0:1], aa[:, :, None])
            nc.vector.tensor_scalar_mul(
                out=aasn[:, :, 1:2], in0=aa[:, :, None], scalar1=-1.0)
            nc.vector.tensor_mul(
                out=cxd[:], in0=pv[:, :, 0, None].broadcast_to([P, T, 2]),
                in1=aab)
            nc.vector.tensor_mul(
                out=cyn[:], in0=pv[:, :, 1, None].broadcast_to([P, T, 2]),
                in1=aasn[:])

            for g in range(NG):
                g0, g1 = g * G, (g + 1) * G
                sc = iopool.tile([P, G, ROW], BF16, tag=f"sc{g}", name=f"sc{g}")
                nc.sync.dma_start(out=sc[:], in_=nbr[:, g0:g1])

                qxy4 = sc[:, :, 0:2 * K].rearrange(
                    "p g (k two) -> p g k two", two=2)
                qz4 = sc[:, :, 2 * K:ROW].rearrange(
                    "p g (h two) -> p g h two", two=2)
                SHP2 = [P, G, K, 2]
                bnxy = nxy2[:, g0:g1, None, :].broadcast_to(SHP2)
                bnz = nzd[:, g0:g1, None, :].broadcast_to([P, G, K // 2, 2])
                bcxx = cxd[:, g0:g1, None, :].broadcast_to(SHP2)
                bcyn = cyn[:, g0:g1, None, :].broadcast_to(SHP2)

                def mtile(tag):
                    return tmppool.tile([P, F2], BF16, tag=tag, name=f"{tag}_{g}")

                def v4(tile):
                    return tile[:].rearrange(
                        "p (g k two) -> p g k two", g=G, two=2)

                dxy = mtile("dxy")
                d4 = v4(dxy)
                nc.vector.tensor_add(out=d4, in0=qxy4, in1=bnxy)
                # oz = qz - pz in place
                nc.vector.tensor_add(out=qz4, in0=qz4, in1=bnz)

                m14 = mtile("m14")    # (dx*cx, dy*cx)
                m23s = mtile("m23s")  # (dy*cy, -dx*cy)  [swapped d pairs]
                nc.vector.tensor_mul(out=v4(m14), in0=d4, in1=bcxx)
                nc.vector.tensor_mul(
                    out=v4(m23s), in0=d4[:, :, :, ::-1], in1=bcyn)

                if g == NG - 1:
                    # last group: sum on DVE in two halves, each stored via
                    # its own DMA ring, so the final transfers overlap the
                    # remaining compute and each other instead of draining
                    # through the serial PE -> ACT path
                    H = G // 2
                    m14v, m23v = v4(m14), v4(m23s)
                    qa = sc[:, 0:H, 0:2 * K].rearrange(
                        "p g (k two) -> p g k two", two=2)
                    qb = sc[:, H:G, 0:2 * K].rearrange(
                        "p g (k two) -> p g k two", two=2)
                    nc.vector.tensor_add(
                        out=qa, in0=m14v[:, 0:H], in1=m23v[:, 0:H])
                    nc.sync.dma_start(
                        out=outr[:, g0:g0 + H], in_=sc[:, 0:H])
                    nc.vector.tensor_add(
                        out=qb, in0=m14v[:, H:G], in1=m23v[:, H:G])
                    nc.scalar.dma_start(
                        out=outr[:, g0 + H:g1], in_=sc[:, H:G])
                    continue

                # oxy = m14 + m23s on TensorE (identity matmul, PSUM fp32
                # accumulation); ACT copies PSUM back into sc
                ps = pspool.tile([P, F2], F32, tag="ps", name=f"ps{g}")
                for c0 in range(0, F2, MM):
                    nc.tensor.matmul(
                        ps[:, c0:c0 + MM], id_sb[:], m14[:, c0:c0 + MM],
                        start=True, stop=False,
                    )
                    nc.tensor.matmul(
                        ps[:, c0:c0 + MM], id_sb[:], m23s[:, c0:c0 + MM],
                        start=False, stop=True,
                    )
                nc.scalar.activation(
                    out=sc[:, :, 0:2 * K],
                    in_=ps[:].rearrange("p (g f) -> p g f", g=G),
                    func=AF.Copy,
                )

                nc.scalar.dma_start(out=outr[:, g0:g1], in_=sc[:])

    nc.compile()
    return nc


def _get_nc():
    if "nc" not in _CACHE:
        _CACHE["nc"] = _build_nc()
    return _CACHE["nc"]


def make_in_maps(points, neighborhoods):
    pts = np.ascontiguousarray(np.asarray(points, dtype=np.float32))
    nb = np.asarray(neighborhoods, dtype=np.float32)
    assert pts.shape == (B, N, 3), pts.shape
    assert nb.shape == (B, N, K, 3), nb.shape

    nb16 = nb.astype(bfloat16)  # [B, N, K, 3]
    ident = np.eye(P, dtype=bfloat16)

    in_maps = []
    for c in range(NCORES):
        sl = slice(c * NLOC, (c + 1) * NLOC)
        a = nb16[:, sl].reshape(B, S, T, K, 3)
        xy = a[..., 0:2].reshape(B, S, T, 2 * K)  # (qx,qy) interleaved
        z = a[..., 2]                             # (B,S,T,K)
        nbh = np.concatenate([xy, z], axis=3).reshape(P, T, ROW)
        in_maps.append({
            "points": np.ascontiguousarray(pts[:, sl]),
            "nbh": np.ascontiguousarray(nbh),
            "ident": ident,
        })
    return in_maps


def kernel(points, neighborhoods):
    in_maps = make_in_maps(points, neighborhoods)
    res = run_bass_kernel_spmd(_get_nc(), in_maps, list(range(NCORES))).results
    parts = []
    for c in range(NCORES):
        o = np.asarray(res[c]["outh"]).reshape(B, S, T, ROW)
        oxy = o[..., :2 * K].reshape(B, S, T, K, 2)
        oz = o[..., 2 * K:]
        full = np.concatenate([oxy, oz[..., None]], axis=4)  # (B,S,T,K,3)
        parts.append(full.reshape(B, NLOC, K, 3))
    return np.concatenate(parts, axis=1).astype(np.float32)


# revision 23
# speedup vs baseline: 1.1333x; 1.1333x over previous
"""Trainium2 Bass kernel for NeighborhoodNormalization.

Math: the reference builds a per-point homogeneous transform
T = [[ux,-uy,0,px],[uy,ux,0,py],[0,0,1,pz],[0,0,0,1]] (u = p/||p||),
inverts it, and applies it to 64 neighbors per point.  Closed form with
r2 = px^2+py^2, n = ||p||, a = n/r2, cx = px*a, cy = py*a, d = q - p:

    out.x =  cx*dx + cy*dy
    out.y =  cx*dy - cy*dx
    out.z =  dz

Strategy (memory-bound, tolerance 2e-2 allows bf16):
  * Host converts neighborhoods to bf16 in a per-core SBUF layout
    [128 partitions, 128 cols, 192] whose inner 192 elems are the 64
    (qx,qy) interleaved pairs followed by the 64 qz -> halves HBM traffic,
    every group DMA is one contiguous 6 KB run per partition (415 GB/s
    measured), and the xy math runs as MERGED 2048-elem pair ops:
        dxy  = qxy + (-px,-py)        m14  = dxy * (cx,cx)
        m23s = swap(dxy) * (cy,-cy)   oxy  = m14 + m23s
    where swap() is a step -1 inner view (allowed by the DVE 2x perf mode)
    so DVE does 3 wide ops + the in-place z subtraction per group.
  * The final oxy sum runs on the otherwise-idle TensorE as identity-matmul
    PSUM accumulations; ACT copies PSUM back into the group tile, so one
    DMA per group moves all planes out.  The LAST group sums on DVE and
    stores via the idle sync ring so the tail skips the PE->ACT drain.
  * Per-point values enter as [P,T,2] bf16 pair tiles viewed with
    (..)(K:0)(2:1) broadcast APs (keeps the 16-bit 2x DVE mode).
  * GpSimd does NOTHING: its elementwise ops share an SBUF port with DVE
    and degrade DVE ~3.5x (measured); SWDGE DMA-accumulate was abandoned
    (half-rate CCE descriptors that break past 2048 elems/run).

Sharding: pure data parallel over N=8192 points across 8 cores.
Per-core layout: 16384 points = 128 partitions x 128 columns, partition
p = b*8 + s holds points with local n = s*128 + t.
"""

import sys

if "/opt/trn_rl_repo" not in sys.path:
    sys.path.insert(0, "/opt/trn_rl_repo")

import numpy as np
from ml_dtypes import bfloat16

import concourse.bass as bass
import concourse.bacc as bacc
import concourse.mybir as mybir
from concourse.bass import MemorySpace
from concourse.tile import TileContext
from concourse.bass_utils import run_bass_kernel_spmd

B = 16
N = 8192
K = 64
NCORES = 8
NLOC = N // NCORES  # 1024 points per core
P = 128             # SBUF partitions
S = NLOC // P       # 8 partition sub-blocks per batch entry
T = (B * NLOC) // P  # 128 point-columns per partition
G = 16              # columns per group
NG = T // G
F2 = 2 * G * K      # xy elems per group (2048)
FZ = G * K          # z elems per group (1024)
ROW = 3 * K         # 192 elems per column (128 xy-interleaved + 64 z)
MM = 512            # moving free-dim max per matmul

F32 = mybir.dt.float32
BF16 = mybir.dt.bfloat16
OP = mybir.AluOpType
AF = mybir.ActivationFunctionType

_CACHE = {}


def _build_nc():
    nc = bacc.Bacc(None, target_bir_lowering=False)

    pts = nc.declare_dram_parameter("points", [B, NLOC, 3], F32, isOutput=False)
    nb = nc.declare_dram_parameter("nbh", [P, T, ROW], BF16, isOutput=False)
    ident = nc.declare_dram_parameter("ident", [P, P], BF16, isOutput=False)
    out = nc.declare_dram_parameter("outh", [P, T, ROW], BF16, isOutput=True)

    nbr = nb[:]
    outr = out[:]
    ptsr = pts[:].rearrange("b (s t) c -> (b s) (t c)", s=S)

    with TileContext(nc) as tc:
        with tc.tile_pool(name="const", bufs=1) as cpool, \
             tc.tile_pool(name="io", bufs=1) as iopool, \
             tc.tile_pool(name="tmp", bufs=5) as tmppool, \
             tc.tile_pool(name="ps", bufs=2, space=MemorySpace.PSUM) as pspool:

            pts_sb = cpool.tile([P, T * 3], F32, tag="pts")
            nc.sync.dma_start(out=pts_sb[:], in_=ptsr)
            id_sb = cpool.tile([P, P], BF16, tag="id")
            nc.sync.dma_start(out=id_sb[:], in_=ident[:])

            pv = pts_sb[:].rearrange("p (t c) -> p t c", c=3)
            px = pv[:, :, 0]
            py = pv[:, :, 1]
            pz = pv[:, :, 2]

            def ctile(tag, dtype=F32, shape=None):
                return cpool.tile(shape or [P, T], dtype, tag=tag, name=tag)

            # (-px, -py) pairs and (-pz, -pz) pairs (need only pts)
            nxy2 = ctile("nxy2", BF16, [P, T, 2])
            nzd = ctile("nzd", BF16, [P, T, 2])
            nc.vector.tensor_scalar_mul(
                out=nxy2[:], in0=pv[:, :, 0:2], scalar1=-1.0)
            nc.vector.tensor_scalar_mul(
                out=nzd[:], in0=pv[:, :, 2, None].broadcast_to([P, T, 2]),
                scalar1=-1.0,
            )

            t1 = ctile("t1")
            t2 = ctile("t2")
            r2 = ctile("r2")
            n2 = ctile("n2")
            nn = ctile("nn")
            ir2 = ctile("ir2")
            aa = ctile("aa")

            nc.vector.tensor_mul(out=t1[:], in0=px, in1=px)
            nc.vector.tensor_mul(out=t2[:], in0=py, in1=py)
            nc.vector.tensor_add(out=r2[:], in0=t1[:], in1=t2[:])
            nc.vector.tensor_mul(out=t1[:], in0=pz, in1=pz)
            nc.vector.tensor_add(out=n2[:], in0=r2[:], in1=t1[:])
            nc.scalar.sqrt(out=nn[:], in_=n2[:])
            # ~18 correct bits, 5x faster than reciprocal(); bf16 keeps 8.
            # randn points make r2=0/denorm/inf impossible.
            nc.vector.reciprocal_approx_fast(out=ir2[:], in_=r2[:])
            nc.vector.tensor_mul(out=aa[:], in0=nn[:], in1=ir2[:])

            # coefficient pair tiles, written directly as bf16 pairs
            cxd = ctile("cxd", BF16, [P, T, 2])   # (cx, cx)
            cyn = ctile("cyn", BF16, [P, T, 2])   # (cy, -cy)
            aasn = ctile("aasn", F32, [P, T, 2])  # (aa, -aa)
            aab = aa[:, :, None].broadcast_to([P, T, 2])
            nc.vector.tensor_copy(aasn[:, :, 0:1], aa[:, :, None])
            nc.vector.tensor_scalar_mul(
                out=aasn[:, :, 1:2], in0=aa[:, :, None], scalar1=-1.0)
            nc.vector.tensor_mul(
                out=cxd[:], in0=pv[:, :, 0, None].broadcast_to([P, T, 2]),
                in1=aab)
            nc.vector.tensor_mul(
                out=cyn[:], in0=pv[:, :, 1, None].broadcast_to([P, T, 2]),
                in1=aasn[:])

            for g in range(NG):
                g0, g1 = g * G, (g + 1) * G
                sc = iopool.tile([P, G, ROW], BF16, tag=f"sc{g}", name=f"sc{g}")
                nc.sync.dma_start(out=sc[:], in_=nbr[:, g0:g1])

                qxy4 = sc[:, :, 0:2 * K].rearrange(
                    "p g (k two) -> p g k two", two=2)
                qz4 = sc[:, :, 2 * K:ROW].rearrange(
                    "p g (h two) -> p g h two", two=2)
                SHP2 = [P, G, K, 2]
                bnxy = nxy2[:, g0:g1, None, :].broadcast_to(SHP2)
                bnz = nzd[:, g0:g1, None, :].broadcast_to([P, G, K // 2, 2])
                bcxx = cxd[:, g0:g1, None, :].broadcast_to(SHP2)
                bcyn = cyn[:, g0:g1, None, :].broadcast_to(SHP2)

                def mtile(tag):
                    return tmppool.tile([P, F2], BF16, tag=tag, name=f"{tag}_{g}")

                def v4(tile):
                    return tile[:].rearrange(
                        "p (g k two) -> p g k two", g=G, two=2)

                dxy = mtile("dxy")
                d4 = v4(dxy)
                nc.vector.tensor_add(out=d4, in0=qxy4, in1=bnxy)
                # oz = qz - pz in place
                nc.vector.tensor_add(out=qz4, in0=qz4, in1=bnz)

                m14 = mtile("m14")    # (dx*cx, dy*cx)
                m23s = mtile("m23s")  # (dy*cy, -dx*cy)  [swapped d pairs]
                nc.vector.tensor_mul(out=v4(m14), in0=d4, in1=bcxx)
                nc.vector.tensor_mul(
                    out=v4(m23s), in0=d4[:, :, :, ::-1], in1=bcyn)

                if g == NG - 1:
                    # last group: sum on DVE, store on the idle sync ring so
                    # the tail skips the serial PE -> ACT drain
                    nc.vector.tensor_add(out=qxy4, in0=v4(m14), in1=v4(m23s))
                    nc.sync.dma_start(out=outr[:, g0:g1], in_=sc[:])
                    continue

                # oxy = m14 + m23s on TensorE (identity matmul, PSUM fp32
                # accumulation); ACT copies PSUM back into sc
                ps = pspool.tile([P, F2], F32, tag="ps", name=f"ps{g}")
                for c0 in range(0, F2, MM):
                    nc.tensor.matmul(
                        ps[:, c0:c0 + MM], id_sb[:], m14[:, c0:c0 + MM],
                        start=True, stop=False,
                    )
                    nc.tensor.matmul(
                        ps[:, c0:c0 + MM], id_sb[:], m23s[:, c0:c0 + MM],
                        start=False, stop=True,
                    )
                nc.scalar.activation(
                    out=sc[:, :, 0:2 * K],
                    in_=ps[:].rearrange("p (g f) -> p g f", g=G),
                    func=AF.Copy,
                )

                nc.scalar.dma_start(out=outr[:, g0:g1], in_=sc[:])

    nc.compile()
    return nc


def _get_nc():
    if "nc" not in _CACHE:
        _CACHE["nc"] = _build_nc()
    return _CACHE["nc"]


def make_in_maps(points, neighborhoods):
    pts = np.ascontiguousarray(np.asarray(points, dtype=np.float32))
    nb = np.asarray(neighborhoods, dtype=np.float32)
    assert pts.shape == (B, N, 3), pts.shape
    assert nb.shape == (B, N, K, 3), nb.shape

    nb16 = nb.astype(bfloat16)  # [B, N, K, 3]
    ident = np.eye(P, dtype=bfloat16)

    in_maps = []
    for c in range(NCORES):
        sl = slice(c * NLOC, (c + 1) * NLOC)
        a = nb16[:, sl].reshape(B, S, T, K, 3)
        xy = a[..., 0:2].reshape(B, S, T, 2 * K)  # (qx,qy) interleaved
        z = a[..., 2]                             # (B,S,T,K)
        nbh = np.concatenate([xy, z], axis=3).reshape(P, T, ROW)
        in_maps.append({
            "points": np.ascontiguousarray(pts[:, sl]),
            "nbh": np.ascontiguousarray(nbh),
            "ident": ident,
        })
    return in_maps


def kernel(points, neighborhoods):
    in_maps = make_in_maps(points, neighborhoods)
    res = run_bass_kernel_spmd(_get_nc(), in_maps, list(range(NCORES))).results
    parts = []
    for c in range(NCORES):
        o = np.asarray(res[c]["outh"]).reshape(B, S, T, ROW)
        oxy = o[..., :2 * K].reshape(B, S, T, K, 2)
        oz = o[..., 2 * K:]
        full = np.concatenate([oxy, oz[..., None]], axis=4)  # (B,S,T,K,3)
        parts.append(full.reshape(B, NLOC, K, 3))
    return np.concatenate(parts, axis=1).astype(np.float32)
